# revision 5
# baseline (speedup 1.0000x reference)
"""Axial (frame-local) multi-head attention for Trainium2, 8-core SPMD.

Problem: x:[2,8192,512] -> qkv proj -> per-(batch,head,frame) attention over
n=1024 tokens -> out proj. B=2, f=8 frames, h=8 heads, d=64.

Sharding: the 16 (batch, frame) pairs are embarrassingly parallel; each of
the 8 cores handles 2 pairs end-to-end (weights replicated). Host
pre-transposes x (bf16) so every on-chip matmul operand is naturally laid
out and every stationary operand is bf16 (background weight loads).

Per-core pipeline (pair p, head-pair hp = heads 2hp,2hp+1):
  A:  qkT = [Wq*scale | Wk]^T @ xT   [1024ch, 1024tok] bf16 (ch-major)
      vvh = xT^T @ Wv                [1024tok, 8x(64v|1ones)] bf16
      (v eviction scatters per-head 64 cols at stride 65; col h*65+64 is a
      memset ones column so each av matmul also accumulates the softmax
      denominator -- no separate den ones-matmuls on the PE at all.)
  B:  per (hp, jt): simA/simD psum [128j, 1024 = 1 head x 1024 i] each;
      exp: ACT one head (single N=1024 activation), DVE the other via a
      one-op Schraudolph (int16(a*x+b) bit-written into the bf16 tile);
      av: per head, 2 matmuls M=65 (64 v-dims + ones) K=128 N=512 into
      po0/po1 [65, 1024] -- partition 64 accumulates sum(exp) = den.
      Per hp finish(): po evicted [65,1024] to SBUF (frees banks fast);
      den rows DMA to DRAM [2,1024] -> reorder read [128,16] -> one
      reciprocal -> reorder write -> 2 broadcast DMAs into one [128,1024]
      den tile; headB numerators DMA-shifted SBUF->SBUF to partitions
      64:128; two gpsimd muls produce otn (muls sit on gpsimd so the DMA
      round-trip wait cannot head-of-line-block ACT/DVE).
  C:  y = otn^T @ Wout (+ bias via K=1 ones matmul), all-bf16 operands.

PSUM (8 banks): sA 2 + sD 2 + po0 2 + po1 2. A and C rotate over the same
four tags with half-used [128,1024] tiles.
"""
import json
import numpy as np
from contextlib import ExitStack

import concourse.bass as bass
import concourse.tile as tile
import concourse.mybir as mybir
from concourse.bass_utils import run_bass_kernel_spmd

F32 = mybir.dt.float32
BF16 = mybir.dt.bfloat16
I16 = mybir.dt.int16
AF = mybir.ActivationFunctionType
ALU = mybir.AluOpType

B, NTOT, DIM = 2, 8192, 512
H, D, F = 8, 64, 8
N = NTOT // F            # 1024 tokens per frame
SCALE = D ** -0.5
NP = 2                   # (batch, frame) pairs per core
TOK = NP * N             # 2048 tokens per core

# Schraudolph exp in bf16 bit space: bf16_bits(exp(x)) ~ round(A16*x + B16)
A16 = 2.0 ** 7 / np.log(2.0)      # 184.6650
B16 = 16256.0 - 7.5               # 0x3F80 minus RMS-optimal shift


def _legalize_waits(bir: bytes) -> bytes:
    """TRN2 instructions carry a single HW wait slot and this walrus build
    refuses to split multi-wait instructions; hoist extra waits onto NoOps
    inserted just before, on the same engine stream."""
    j = json.loads(bir)
    ctr = 0
    for fn in j["functions"]:
        for blk in fn["blocks"]:
            out = []
            for inst in blk["instructions"]:
                si = inst.get("sync_info")
                if si:
                    waits = si.get("on_wait") or []
                    if len(waits) > 1:
                        for w in waits[:-1]:
                            ctr += 1
                            nop = {
                                "engine": inst["engine"],
                                "ins": [], "outs": [],
                                "name": f"I-waitfix-{ctr}",
                                "opcode": "NoOp",
                                "sync_info": {"on_update": [], "on_wait": [w]},
                            }
                            if "debug" in inst:
                                nop["debug"] = inst["debug"]
                            out.append(nop)
                        si["on_wait"] = waits[-1:]
                out.append(inst)
            blk["instructions"] = out
    return json.dumps(j).encode()


def build(with_bias=True):
    nc = bass.Bass(trn_type="TRN2")
    xt = nc.dram_tensor("xt", [DIM, TOK], BF16, kind="ExternalInput")
    wqk = nc.dram_tensor("wqk", [DIM, 1024], BF16, kind="ExternalInput")
    wv = nc.dram_tensor("wv", [DIM, 512], BF16, kind="ExternalInput")
    wout = nc.dram_tensor("wout", [DIM, 512], BF16, kind="ExternalInput")
    bout = nc.dram_tensor("bout", [1, 512], BF16, kind="ExternalInput")
    y = nc.dram_tensor("y", [TOK, DIM], F32, kind="ExternalOutput")
    rden = [nc.dram_tensor(f"rden{t}", [2, 1024], F32) for t in range(8)]
    rrcp = [nc.dram_tensor(f"rrcp{t}", [2, 1024], F32) for t in range(8)]

    with tile.TileContext(nc) as tc, ExitStack() as ctx:
        const = ctx.enter_context(tc.tile_pool(name="const", bufs=1))
        qk_pool = ctx.enter_context(tc.tile_pool(name="qk", bufs=2))
        vv_pool = ctx.enter_context(tc.tile_pool(name="vv", bufs=2))
        et_pool = ctx.enter_context(tc.tile_pool(name="et", bufs=4))
        otn_pool = ctx.enter_context(tc.tile_pool(name="otn", bufs=2))
        ou_pool = ctx.enter_context(tc.tile_pool(name="ou", bufs=2))
        rd_pool = ctx.enter_context(tc.tile_pool(name="rd", bufs=2))
        y_pool = ctx.enter_context(tc.tile_pool(name="yo", bufs=2))
        den_pool2 = ctx.enter_context(tc.tile_pool(name="dsb", bufs=2))
        ps = ctx.enter_context(tc.tile_pool(name="ps", bufs=1, space="PSUM"))

        # ---- weights / inputs: DMAs ordered by first use, spread across
        # FOUR HWDGE queues (sync+vector carry weights, scalar+gpsimd xt) ----
        wqk_sb = [const.tile([128, 1024], BF16, tag=f"wqk{k}", name=f"wqk{k}")
                  for k in range(4)]
        wv_sb = [const.tile([128, 512], BF16, tag=f"wv{k}", name=f"wv{k}")
                 for k in range(4)]
        wout_sb = [const.tile([128, 512], BF16, tag=f"wout{k}", name=f"wout{k}")
                   for k in range(4)]
        xt_sb = [const.tile([128, TOK], BF16, tag=f"xt{k}", name=f"xt{k}")
                 for k in range(4)]
        bout_sb = const.tile([1, 512], BF16, tag="bout", name="bout")

        wq_q = [nc.sync, nc.sync]
        xt_q = [nc.scalar, nc.gpsimd]
        for k in range(4):
            wq_q[k % 2].dma_start(wqk_sb[k][:, 0:512],
                                  wqk.ap()[k * 128:(k + 1) * 128, 0:512])
            xt_q[k % 2].dma_start(xt_sb[k][:, 0:512],
                                  xt.ap()[k * 128:(k + 1) * 128, 0:512])
        for k in range(4):
            wq_q[k % 2].dma_start(wqk_sb[k][:, 512:1024],
                                  wqk.ap()[k * 128:(k + 1) * 128, 512:1024])
            xt_q[k % 2].dma_start(xt_sb[k][:, 512:N],
                                  xt.ap()[k * 128:(k + 1) * 128, 512:N])
        for k in range(4):
            wq_q[k % 2].dma_start(wv_sb[k][:], wv.ap()[k * 128:(k + 1) * 128, :])
            xt_q[k % 2].dma_start(xt_sb[k][:, N:N + 512],
                                  xt.ap()[k * 128:(k + 1) * 128, N:N + 512])
        for k in range(4):
            wq_q[k % 2].dma_start(wout_sb[k][:], wout.ap()[k * 128:(k + 1) * 128, :])
            xt_q[k % 2].dma_start(xt_sb[k][:, N + 512:TOK],
                                  xt.ap()[k * 128:(k + 1) * 128, N + 512:TOK])
        nc.sync.dma_start(bout_sb[:], bout.ap())

        ones_f = const.tile([128, 128], F32, tag="ones_f", name="ones_f")
        nc.gpsimd.memset(ones_f[:], 1.0)
        ones_b = const.tile([128, 128], BF16, tag="ones_b", name="ones_b")
        nc.gpsimd.memset(ones_b[:], 1.0)
        # warm the ACT exp table set during the startup DMA window
        warm = const.tile([1, 8], F32, tag="warm", name="warm")
        nc.scalar.activation(warm[:], ones_f[0:1, 0:8], AF.Exp)

        S = {0: {}, 1: {}}
        ATAGS = ["sA", "sD", "po0", "po1"]

        def emit_A(pi, tags):
            t0 = pi * N
            # ---- qkT [1024ch, 1024tok], ch-major, bf16 out ----
            S[pi]['qkT'] = qkT = [
                qk_pool.tile([128, N], BF16, tag=f"qkT{c}", name=f"qkT{c}")
                for c in range(8)]
            ci = 0

            def evict(dst, src):
                # alternate eviction engine so neither ACT nor DVE gates
                # the chain pipeline (A runs before B; both are free)
                if ci % 2 == 0:
                    nc.scalar.copy(dst, src)
                else:
                    nc.vector.tensor_copy(dst, src)

            for cht in range(8):
                for nt in range(2):
                    tg = tags[ci % len(tags)]
                    pa = ps.tile([128, 1024], F32, tag=tg, name="pa")
                    for kt in range(4):
                        nc.tensor.matmul(
                            pa[:, 0:512],
                            wqk_sb[kt][:, cht * 128:(cht + 1) * 128],
                            xt_sb[kt][:, t0 + nt * 512:t0 + (nt + 1) * 512],
                            start=(kt == 0), stop=(kt == 3))
                    evict(qkT[cht][:, nt * 512:(nt + 1) * 512], pa[:, 0:512])
                    ci += 1

            # ---- v tok-major bf16, per-head [64 v | 1 ones] at stride 65 ----
            S[pi]['vv'] = vvh = [
                vv_pool.tile([128, 8 * 65], BF16, tag=f"vv{t}", name=f"vv{t}")
                for t in range(8)]
            for tt in range(8):
                tg = tags[ci % len(tags)]
                pv = ps.tile([128, 1024], F32, tag=tg, name="pv")
                for kt in range(4):
                    nc.tensor.matmul(
                        pv[:, 0:512],
                        xt_sb[kt][:, t0 + tt * 128:t0 + (tt + 1) * 128],
                        wv_sb[kt][:],
                        start=(kt == 0), stop=(kt == 3))
                v3 = vvh[tt][:].rearrange("p (h cc) -> p h cc", h=8)
                nc.gpsimd.memset(v3[:, :, 64:65], 1.0)
                evict(v3[:, :, 0:64],
                      pv[:, 0:512].rearrange("p (h c) -> p h c", h=8))
                ci += 1

        def emit_B():
            # Both pairs interleaved at the head-pair-group level:
            # (hp0,p0),(hp0,p1),(hp1,p0)... -- no phase boundary between
            # pairs, and each group's denominator bounce gets a full
            # group-span of slack before its results are needed.
            for pi in range(2):
                S[pi]['otn'] = [
                    otn_pool.tile([128, N], BF16, tag=f"otn{pi}{t}",
                                  name=f"otn{pi}{t}")
                    for t in range(4)]
            pend = []   # deferred emission closures (1-period software pipe)

            def group(pi, hp):
                qkT = S[pi]['qkT']; vvh = S[pi]['vv']
                otn = S[pi]['otn']
                hA, hB = 2 * hp, 2 * hp + 1
                G = {}

                def avden(jt, ch, et, ets):
                    if 'po' not in G:
                        G['po'] = [ps.tile([128, 1024], F32, tag=f"po{i}",
                                           name=f"po{i}") for i in (0, 1)]
                    po = G['po']
                    # head ch's exp runs on DVE (ets), the other on ACT (et).
                    # Emit the ACT-dependent head's av first: the in-order PE
                    # queue then always has ready work while DVE finishes.
                    src = [ets, et] if ch == 0 else [et, ets]
                    order = [1, 0] if ch == 0 else [0, 1]
                    for h in order:
                        for it in range(2):
                            nc.tensor.matmul(
                                po[h][0:65, it * 512:(it + 1) * 512],
                                vvh[jt][:, (2 * hp + h) * 65:
                                         (2 * hp + h) * 65 + 65],
                                src[h][:, it * 512:(it + 1) * 512],
                                start=(jt == 0), stop=(jt == 7),
                                tile_position=(0, 0))

                def finish():
                    po = G['po']
                    g = pi * 4 + hp
                    # evict numerators+den rows together: frees po banks
                    # fast, off the reciprocal/broadcast path
                    ouA = ou_pool.tile([128, 1024], F32, tag="ouA",
                                       name="ouA")
                    ouB = ou_pool.tile([128, 1024], F32, tag="ouB",
                                       name="ouB")
                    nc.vector.tensor_copy(ouA[0:65, :], po[0][0:65, :])
                    nc.vector.tensor_copy(ouB[0:65, :], po[1][0:65, :])
                    # den rows to DRAM; reorder-read puts i on partitions so
                    # ONE [128,16] reciprocal covers both heads; reorder-write
                    # restores i-major rows for the broadcast reads. All DMAs
                    # ride the sync HWDGE. headB numerators are DMA-shifted
                    # to partitions 64:128 so the muls stay partition-aligned.
                    nc.sync.dma_start(rden[g].ap()[0:1, :], ouA[64:65, :])
                    nc.sync.dma_start(rden[g].ap()[1:2, :], ouB[64:65, :])
                    ouB2 = ou_pool.tile([128, 1024], F32, tag="ouB2",
                                        name="ouB2")
                    nc.sync.dma_start(ouB2[64:128, :], ouB[0:64, :])
                    rin = rd_pool.tile([128, 16], F32, tag="rin", name="rin")
                    for t in range(2):
                        nc.sync.dma_start(
                            rin[t * 64:(t + 1) * 64, :],
                            rden[g].ap()[t:t + 1, :].rearrange(
                                "one (c j) -> (one j) c", j=64))
                    rcpT = rd_pool.tile([128, 16], F32, tag="rcpT",
                                        name="rcpT")
                    nc.vector.reciprocal(rcpT[:], rin[:])
                    for t in range(2):
                        nc.sync.dma_start(
                            rrcp[g].ap()[t:t + 1, :].rearrange(
                                "one (c j) -> (one j) c", j=64),
                            rcpT[t * 64:(t + 1) * 64, :])
                    den_sb = den_pool2.tile([128, 1024], F32, tag="dsb",
                                            name="dsb")
                    nc.sync.dma_start(
                        den_sb[0:64, :],
                        rrcp[g].ap()[0:1, :].broadcast_to([64, 1024]))
                    nc.sync.dma_start(
                        den_sb[64:128, :],
                        rrcp[g].ap()[1:2, :].broadcast_to([64, 1024]))
                    # muls wait on the DMA round-trip: keep them on the
                    # otherwise-idle gpsimd queue (anywhere else they
                    # head-of-line-block that engine's stream)
                    nc.gpsimd.tensor_tensor(otn[hp][0:64, :], ouA[0:64, :],
                                            den_sb[0:64, :], ALU.mult)
                    nc.gpsimd.tensor_tensor(otn[hp][64:128, :],
                                            ouB2[64:128, :],
                                            den_sb[64:128, :], ALU.mult)

                qt, kt_ = hp, 4 + hp
                for jt in range(8):
                    # sim PSUM split by exp consumer: ACT and DVE must not
                    # read the same tile (same-tile readers are serialized
                    # by the scheduler's semaphore coalescing). One head's
                    # 1024 logits go to each engine (phi=0.5 exp split).
                    simA = ps.tile([128, 1024], F32, tag="sA", name="simA")
                    simD = ps.tile([128, 1024], F32, tag="sD", name="simD")
                    ch = (jt + hp) % 2   # head whose exp runs on DVE

                    def sim_dst(r):
                        t = simD if r // 2 == ch else simA
                        return t[:, (r % 2) * 512:(r % 2 + 1) * 512]

                    for it in range(2):
                        nc.tensor.matmul(
                            sim_dst(it),
                            qkT[kt_][0:64, jt * 128:(jt + 1) * 128],
                            qkT[qt][0:64, it * 512:(it + 1) * 512],
                            start=True, stop=True, tile_position=(0, 0))
                        nc.tensor.matmul(
                            sim_dst(2 + it),
                            qkT[kt_][64:128, jt * 128:(jt + 1) * 128],
                            qkT[qt][64:128, it * 512:(it + 1) * 512],
                            start=True, stop=True, tile_position=(64, 0))
                    et = et_pool.tile([128, 1024], BF16, tag="et", name="et")
                    ets = et_pool.tile([128, 1024], BF16, tag="ets",
                                       name="ets")
                    # exp: DVE does one head via the Schraudolph bit-trick,
                    # ACT the other head in a single activation
                    nc.vector.tensor_scalar(
                        ets[:].bitcast(I16), simD[:],
                        A16, B16, ALU.mult, ALU.add)
                    nc.scalar.activation(et[:], simA[:], AF.Exp)
                    # av of jt emits one period late so the in-order PE
                    # queue always has ready work while exp(jt) runs
                    pend.append((lambda j=jt, c=ch, e=et, es=ets:
                                 avden(j, c, e, es)))
                    if len(pend) > 3:
                        pend.pop(0)()
                pend.append(finish)

            # pair-interleaved, but pair0's last group runs two groups
            # before the end so C0 can weave into B's tail
            for pi, hp in [(0, 0), (1, 0), (0, 1), (1, 1), (0, 2), (0, 3),
                           (1, 2), (1, 3)]:
                group(pi, hp)
            while pend:
                pend.pop(0)()

        def emit_C(pi, tags, evict):
            # Two passes: kt0-2 partial chains run as soon as the first
            # three head-pairs' otn exist (weavable into B's tail); only
            # one matmul + one DVE add per token block depends on the
            # last head-pair, shrinking the end-of-kernel serial tail.
            otn = S[pi]['otn']
            part = []
            for tt in range(8):
                tg = tags[tt % len(tags)]
                py = ps.tile([128, 1024], F32, tag=tg, name="py")
                if with_bias:
                    nc.tensor.matmul(py[:, 0:512], ones_b[0:1, :], bout_sb[:],
                                     start=True, stop=False,
                                     tile_position=(0, 0))
                for kt in range(3):
                    nc.tensor.matmul(
                        py[:, 0:512],
                        otn[kt][:, tt * 128:(tt + 1) * 128],
                        wout_sb[kt][:],
                        start=(not with_bias and kt == 0), stop=(kt == 2))
                yp = y_pool.tile([128, 512], F32, tag="yp", name="yp",
                                 bufs=8)
                # all evictions on DVE: the ACT queue at the tail carries
                # the y DMAs, and an eviction queued behind them stalls
                # every 4th chain's PSUM bank reuse
                evict(yp[:], py[:, 0:512])
                part.append(yp)
            for tt in range(8):
                tg = tags[tt % len(tags)]
                py = ps.tile([128, 1024], F32, tag=tg, name="py")
                nc.tensor.matmul(
                    py[:, 0:512],
                    otn[3][:, tt * 128:(tt + 1) * 128],
                    wout_sb[3][:],
                    start=True, stop=True)
                ysb = y_pool.tile([128, 512], F32, tag="ysb", name="ysb",
                                  bufs=8)
                nc.vector.tensor_tensor(ysb[:], part[tt][:], py[:, 0:512],
                                        ALU.add)
                # y rides the ACT HWDGE: the sync queue may still be
                # draining den bounces that wait multi-us on reciprocals
                nc.scalar.dma_start(
                    y.ap()[pi * N + tt * 128:pi * N + (tt + 1) * 128, :],
                    ysb[:])

        def ev_dve(dst, src):
            nc.vector.tensor_copy(dst, src)

        # Both projections run up-front at full PE density (B-phase PE has
        # no slack to absorb them); B is then ACT/PE balanced, with both
        # pairs' head-pair groups interleaved in one phase.
        emit_A(0, ATAGS)
        emit_A(1, ATAGS)
        emit_B()
        emit_C(0, ATAGS, ev_dve)       # starts inside B's tail
        emit_C(1, ATAGS, ev_dve)       # tail: all banks free again

    _orig = nc.to_json_bytes
    nc.to_json_bytes = lambda: _legalize_waits(_orig())
    return nc


_NC_CACHE = []
_last_in_maps = None


def kernel(**inputs) -> np.ndarray:
    import ml_dtypes
    BF = ml_dtypes.bfloat16
    x = np.ascontiguousarray(np.asarray(inputs["x"], dtype=np.float32))
    W_qkv = np.asarray(inputs["W_qkv"], dtype=np.float32)
    W_out = np.ascontiguousarray(np.asarray(inputs["W_out"], dtype=np.float32))
    b_out = np.ascontiguousarray(np.asarray(inputs["b_out"], dtype=np.float32))
    f = int(np.asarray(inputs["f"]))
    assert f == F and x.shape == (B, NTOT, DIM)

    Wqk = np.ascontiguousarray(np.concatenate(
        [W_qkv[:, :512] * SCALE, W_qkv[:, 512:1024]], axis=1).astype(BF))
    Wv = np.ascontiguousarray(W_qkv[:, 1024:1536].astype(BF))
    Wo = np.ascontiguousarray(W_out.astype(BF))
    bo = np.ascontiguousarray(b_out.reshape(1, 512).astype(BF))

    with_bias = bool(np.any(b_out))
    key = with_bias
    if not _NC_CACHE or _NC_CACHE[0][0] != key:
        _NC_CACHE.clear()
        _NC_CACHE.append((key, build(with_bias)))
    nc = _NC_CACHE[0][1]

    in_maps = []
    for core in range(8):
        pairs = (2 * core, 2 * core + 1)
        xT = np.concatenate(
            [x[p // F, (p % F) * N:(p % F + 1) * N, :].T for p in pairs], axis=1)
        in_maps.append({
            "xt": np.ascontiguousarray(xT.astype(BF)),
            "wqk": Wqk, "wv": Wv, "wout": Wo, "bout": bo,
        })

    global _last_in_maps
    _last_in_maps = in_maps
    try:
        res = run_bass_kernel_spmd(nc, in_maps, list(range(8)))
    except Exception:
        # transient NRT_EXEC_UNIT_UNRECOVERABLE occasionally hits the first
        # submission after a fresh compile; one retry has always cleared it
        import time
        time.sleep(10)
        res = run_bass_kernel_spmd(nc, in_maps, list(range(8)))

    out = np.zeros((B, NTOT, DIM), dtype=np.float32)
    for core in range(8):
        yc = res.results[core]["y"]
        for pi, p in enumerate((2 * core, 2 * core + 1)):
            out[p // F, (p % F) * N:(p % F + 1) * N, :] = yc[pi * N:(pi + 1) * N]
    return out


# revision 8
# speedup vs baseline: 1.1417x; 1.1417x over previous
"""Axial (frame-local) multi-head attention for Trainium2, 8-core SPMD.

Problem: x:[2,8192,512] -> qkv proj -> per-(batch,head,frame) attention over
n=1024 tokens -> out proj. B=2, f=8 frames, h=8 heads, d=64.

Sharding: the 16 (batch, frame) pairs are embarrassingly parallel; each of
the 8 cores handles 2 pairs end-to-end (weights replicated). Host
pre-transposes x (bf16) so every on-chip matmul operand is naturally laid
out and every stationary operand is bf16 (FWL + background weight loads --
fp32 weight loads cannot be pulled ahead and serialize ~190ns per matmul).

Per-core pipeline (pair p, head-pair hp = heads 2hp,2hp+1):
  A:  qkT = [Wq*scale | Wk]^T @ xT   [1024ch, 1024tok] bf16 (ch-major)
      vv  = xT^T @ Wv                [1024tok, 512]    bf16 (tok-major)
  B:  per (hp, jt): simAB psum [128j, 2048 = 2 heads x 1024 i] via 4
      row-group-packed K=64 matmuls (2 heads concurrent);
      exp: ACT does 3 of 4 512-col chunks in ONE N=1536 activation, DVE
      does the 4th with a one-op Schraudolph (int16(a*x+b) bit-written
      into the bf16 et tile); av/den matmuls that read ACT chunks are
      emitted before those reading the DVE chunk (PE queue is in-order);
      av: two col-tiled M=64 matmuls (both heads concurrent);
      den: four col-tiled M=1 ones-matmuls accumulate the softmax
      denominators into one PSUM bank at partitions 0/32/64/96.
      Per hp: po evicted to SBUF immediately (frees the PSUM bank off the
      reciprocal critical path); den bank -> StreamTranspose -> one
      strided-free reciprocal [128,16] -> reordering DMAs to DRAM [4,512]
      -> broadcast DMAs -> normalize multiply into bf16 otn.
  C:  y = otn^T @ Wout (+ bias via K=1 ones matmul), all-bf16 operands.

PSUM (8 banks): simAB 4 + po0 1 + po1 1 + pden 1 + pa 1. Stage A of
pair 0 and C of pair 1 run when B is absent and rotate over all 4
single-bank tags; A1/C0 run inside B's span on the pa bank only.
Evictions: ACT engine when it is idle (A0, C1), DVE otherwise.
"""
import json
import numpy as np
from contextlib import ExitStack

import concourse.bass as bass
import concourse.tile as tile
import concourse.mybir as mybir
from concourse.bass_utils import run_bass_kernel_spmd

F32 = mybir.dt.float32
BF16 = mybir.dt.bfloat16
I16 = mybir.dt.int16
AF = mybir.ActivationFunctionType
ALU = mybir.AluOpType

B, NTOT, DIM = 2, 8192, 512
H, D, F = 8, 64, 8
N = NTOT // F            # 1024 tokens per frame
SCALE = D ** -0.5
NP = 2                   # (batch, frame) pairs per core
TOK = NP * N             # 2048 tokens per core

# Schraudolph exp in bf16 bit space: bf16_bits(exp(x)) ~ round(A16*x + B16)
A16 = 2.0 ** 7 / np.log(2.0)      # 184.6650
B16 = 16256.0 - 7.5               # 0x3F80 minus RMS-optimal shift


def _legalize_waits(bir: bytes) -> bytes:
    """TRN2 instructions carry a single HW wait slot and this walrus build
    refuses to split multi-wait instructions; hoist extra waits onto NoOps
    inserted just before, on the same engine stream."""
    j = json.loads(bir)
    ctr = 0
    for fn in j["functions"]:
        for blk in fn["blocks"]:
            out = []
            for inst in blk["instructions"]:
                si = inst.get("sync_info")
                if si:
                    waits = si.get("on_wait") or []
                    if len(waits) > 1:
                        for w in waits[:-1]:
                            ctr += 1
                            nop = {
                                "engine": inst["engine"],
                                "ins": [], "outs": [],
                                "name": f"I-waitfix-{ctr}",
                                "opcode": "NoOp",
                                "sync_info": {"on_update": [], "on_wait": [w]},
                            }
                            if "debug" in inst:
                                nop["debug"] = inst["debug"]
                            out.append(nop)
                        si["on_wait"] = waits[-1:]
                out.append(inst)
            blk["instructions"] = out
    return json.dumps(j).encode()


def build(with_bias=True):
    nc = bass.Bass(trn_type="TRN2")
    xt = nc.dram_tensor("xt", [DIM, TOK], BF16, kind="ExternalInput")
    wqk = nc.dram_tensor("wqk", [DIM, 1024], BF16, kind="ExternalInput")
    wv = nc.dram_tensor("wv", [DIM, 512], BF16, kind="ExternalInput")
    wout = nc.dram_tensor("wout", [DIM, 512], BF16, kind="ExternalInput")
    bout = nc.dram_tensor("bout", [1, 512], BF16, kind="ExternalInput")
    y = nc.dram_tensor("y", [TOK, DIM], F32, kind="ExternalOutput")
    rscr = [nc.dram_tensor(f"rscr{t}", [4, 512], F32) for t in range(8)]

    with tile.TileContext(nc) as tc, ExitStack() as ctx:
        const = ctx.enter_context(tc.tile_pool(name="const", bufs=1))
        qk_pool = ctx.enter_context(tc.tile_pool(name="qk", bufs=2))
        vv_pool = ctx.enter_context(tc.tile_pool(name="vv", bufs=2))
        et_pool = ctx.enter_context(tc.tile_pool(name="et", bufs=4))
        otn_pool = ctx.enter_context(tc.tile_pool(name="otn", bufs=2))
        ou_pool = ctx.enter_context(tc.tile_pool(name="ou", bufs=4))
        rd_pool = ctx.enter_context(tc.tile_pool(name="rd", bufs=2))
        y_pool = ctx.enter_context(tc.tile_pool(name="yo", bufs=2))
        den_pool2 = ctx.enter_context(tc.tile_pool(name="dsb", bufs=4))
        ps = ctx.enter_context(tc.tile_pool(name="ps", bufs=1, space="PSUM"))

        # ---- weights / inputs: DMAs ordered by first use, split across
        # the two HWDGE queues (SP carries wqk, ACT carries xt) ----
        wqk_sb = [const.tile([128, 1024], BF16, tag=f"wqk{k}", name=f"wqk{k}")
                  for k in range(4)]
        wv_sb = [const.tile([128, 512], BF16, tag=f"wv{k}", name=f"wv{k}")
                 for k in range(4)]
        wout_sb = [const.tile([128, 512], BF16, tag=f"wout{k}", name=f"wout{k}")
                   for k in range(4)]
        xt_sb = [const.tile([128, TOK], BF16, tag=f"xt{k}", name=f"xt{k}")
                 for k in range(4)]
        bout_sb = const.tile([1, 512], BF16, tag="bout", name="bout")

        xt_q = [nc.scalar, nc.scalar, nc.gpsimd, nc.gpsimd]
        for k in range(4):
            nc.sync.dma_start(wqk_sb[k][:, 0:512],
                              wqk.ap()[k * 128:(k + 1) * 128, 0:512])
            xt_q[k].dma_start(xt_sb[k][:, 0:512],
                              xt.ap()[k * 128:(k + 1) * 128, 0:512])
        for k in range(4):
            nc.sync.dma_start(wqk_sb[k][:, 512:1024],
                              wqk.ap()[k * 128:(k + 1) * 128, 512:1024])
            xt_q[k].dma_start(xt_sb[k][:, 512:N],
                              xt.ap()[k * 128:(k + 1) * 128, 512:N])
        for k in range(4):
            nc.sync.dma_start(wv_sb[k][:], wv.ap()[k * 128:(k + 1) * 128, :])
            xt_q[k].dma_start(xt_sb[k][:, N:N + 512],
                              xt.ap()[k * 128:(k + 1) * 128, N:N + 512])
        for k in range(4):
            nc.sync.dma_start(wout_sb[k][:], wout.ap()[k * 128:(k + 1) * 128, :])
            xt_q[k].dma_start(xt_sb[k][:, N + 512:TOK],
                              xt.ap()[k * 128:(k + 1) * 128, N + 512:TOK])
        nc.sync.dma_start(bout_sb[:], bout.ap())

        ones_f = const.tile([128, 128], F32, tag="ones_f", name="ones_f")
        nc.gpsimd.memset(ones_f[:], 1.0)
        ones_b = const.tile([128, 128], BF16, tag="ones_b", name="ones_b")
        nc.gpsimd.memset(ones_b[:], 1.0)
        # warm the ACT exp table set during the startup DMA window
        warm = const.tile([1, 8], F32, tag="warm", name="warm")
        nc.scalar.activation(warm[:], ones_f[0:1, 0:8], AF.Exp)

        S = {0: {}, 1: {}}
        ATAGS = ["pa", "po0", "po1", "pden"]

        def emit_A(pi, tags):
            t0 = pi * N
            # ---- qkT [1024ch, 1024tok], ch-major, bf16 out ----
            S[pi]['qkT'] = qkT = [
                qk_pool.tile([128, N], BF16, tag=f"qkT{c}", name=f"qkT{c}")
                for c in range(8)]
            ci = 0

            def evict(dst, src):
                # alternate eviction engine so neither ACT nor DVE gates
                # the chain pipeline (A runs before B; both are free)
                if ci % 2 == 0:
                    nc.scalar.copy(dst, src)
                else:
                    nc.vector.tensor_copy(dst, src)

            for cht in range(8):
                for nt in range(2):
                    tg = tags[ci % len(tags)]
                    pa = ps.tile([128, 512], F32, tag=tg, name="pa")
                    for kt in range(4):
                        nc.tensor.matmul(
                            pa[:],
                            wqk_sb[kt][:, cht * 128:(cht + 1) * 128],
                            xt_sb[kt][:, t0 + nt * 512:t0 + (nt + 1) * 512],
                            start=(kt == 0), stop=(kt == 3))
                    evict(qkT[cht][:, nt * 512:(nt + 1) * 512], pa[:])
                    ci += 1

            # ---- v tok-major bf16 [128 tok, 512 dims] per token block ----
            S[pi]['vv'] = vv = [
                vv_pool.tile([128, 512], BF16, tag=f"vv{t}", name=f"vv{t}")
                for t in range(8)]
            for tt in range(8):
                tg = tags[ci % len(tags)]
                pv = ps.tile([128, 512], F32, tag=tg, name="pv")
                for kt in range(4):
                    nc.tensor.matmul(
                        pv[:],
                        xt_sb[kt][:, t0 + tt * 128:t0 + (tt + 1) * 128],
                        wv_sb[kt][:],
                        start=(kt == 0), stop=(kt == 3))
                evict(vv[tt][:], pv[:])
                ci += 1

        def emit_B():
            # Both pairs interleaved at the head-pair-group level:
            # (hp0,p0),(hp0,p1),(hp1,p0)... -- no phase boundary between
            # pairs, and each group's denominator bounce gets a full
            # group-span of slack before its results are needed.
            for pi in range(2):
                S[pi]['otn'] = [
                    otn_pool.tile([128, N], BF16, tag=f"otn{pi}{t}",
                                  name=f"otn{pi}{t}")
                    for t in range(4)]
            pend = []   # deferred emission closures (1-period software pipe)

            def group(pi, hp):
                qkT = S[pi]['qkT']; vv = S[pi]['vv']
                otn = S[pi]['otn']
                hA, hB = 2 * hp, 2 * hp + 1
                G = {}

                def avden(jt, ch, et, ets):
                    if 'po' not in G:
                        G['po'] = [ps.tile([128, 512], F32, tag=f"po{i}",
                                           name=f"po{i}") for i in (0, 1)]
                        G['pden'] = ps.tile([128, 512], F32, tag="pden",
                                            name="pden")
                    po, pden = G['po'], G['pden']

                    def chunk(r):
                        t = ets if r // 2 == ch else et
                        return t[:, (r % 2) * 512:(r % 2 + 1) * 512]

                    # av: both heads concurrent via column tiling
                    for it in range(2):
                        nc.tensor.matmul(
                            po[it][0:64, :],
                            vv[jt][:, hA * 64:(hA + 1) * 64],
                            chunk(it),
                            start=(jt == 0), stop=(jt == 7),
                            tile_position=(0, 0))
                        nc.tensor.matmul(
                            po[it][64:128, :],
                            vv[jt][:, hB * 64:(hB + 1) * 64],
                            chunk(2 + it),
                            start=(jt == 0), stop=(jt == 7),
                            tile_position=(0, 64))
                    # den: 4 col-tiled ones-matmuls. jt==0 writes M=32
                    # (all 32 rows of each group get the block sum) so the
                    # bank holds no uninitialized words for the transpose.
                    for r in range(4):
                        m = 32 if jt == 0 else 1
                        nc.tensor.matmul(
                            pden[32 * r:32 * r + m, :],
                            ones_b[:, 0:m],
                            chunk(r),
                            start=(jt == 0), stop=(jt == 7),
                            tile_position=(0, 32 * r))

                def finish():
                    po, pden = G['po'], G['pden']
                    # evict numerators first: frees po banks fast, off the
                    # reciprocal/broadcast path
                    ou = [ou_pool.tile([128, 512], F32, tag="ou", name="ou")
                          for _ in range(2)]
                    for it in range(2):
                        nc.vector.tensor_copy(ou[it][:], po[it][:])
                    # denominators: transpose -> strided-free reciprocal
                    # [128,16] -> reordering DMAs to i-major DRAM rows ->
                    # broadcast DMAs -> normalize. DMAs ride the idle sync
                    # HWDGE; the multiplies (which wait on the round-trip)
                    # sit on gpsimd so they cannot head-of-line-block the
                    # Vector stream. The last group runs its muls on the
                    # then-idle DVE (lower latency into the C tail).
                    tp = rd_pool.tile([128, 512], F32, tag="tp", name="tp")
                    nc.vector.transpose(tp[:], pden[:])
                    rcpT = rd_pool.tile([128, 16], F32, tag="rcpT",
                                        name="rcpT")
                    nc.vector.reciprocal(rcpT[:], tp[:, 0:512:32])
                    sc = rscr[pi * 4 + hp]
                    # muls wait on the DMA round-trip: keep them on the
                    # otherwise-idle gpsimd queue (anywhere else they
                    # head-of-line-block that engine's stream; the C-stage
                    # evictions ride DVE right behind B's tail)
                    mul = nc.gpsimd.tensor_tensor
                    for r in range(4):
                        nc.sync.dma_start(
                            sc.ap()[r:r + 1, :].rearrange(
                                "one (c j) -> (one j) c", j=32),
                            rcpT[32 * r:32 * r + 32, :])
                    for it in range(2):
                        den_sb = den_pool2.tile([128, 512], F32, tag="dsb",
                                                name="dsb")
                        nc.sync.dma_start(
                            den_sb[0:64, :],
                            sc.ap()[it:it + 1, :].broadcast_to([64, 512]))
                        nc.sync.dma_start(
                            den_sb[64:128, :],
                            sc.ap()[2 + it:3 + it, :].broadcast_to([64, 512]))
                        mul(otn[hp][:, it * 512:(it + 1) * 512],
                            ou[it][:], den_sb[:], ALU.mult)

                qt, kt_ = hp, 4 + hp
                for jt in range(8):
                    # sim PSUM: FOUR single-bank [128,512] tiles, one per
                    # (engine, it) chunk, each exp'd by its own ACT/DVE op.
                    # The four chunk-loops interleave: sim(jt+1, c) streams
                    # while exp(jt, c') runs, so the sim->exp->sim serial
                    # chain of the 2-tile variant disappears without
                    # needing double-buffered banks.
                    sA = [ps.tile([128, 512], F32, tag=f"sA{i}",
                                  name=f"sA{i}") for i in range(2)]
                    sD = [ps.tile([128, 512], F32, tag=f"sD{i}",
                                  name=f"sD{i}") for i in range(2)]
                    ch = (jt + hp) % 2   # head whose exp runs on DVE

                    def sim_dst(r):
                        t = sD if r // 2 == ch else sA
                        return t[r % 2][:]

                    for it in range(2):
                        nc.tensor.matmul(
                            sim_dst(it),
                            qkT[kt_][0:64, jt * 128:(jt + 1) * 128],
                            qkT[qt][0:64, it * 512:(it + 1) * 512],
                            start=True, stop=True, tile_position=(0, 0))
                        nc.tensor.matmul(
                            sim_dst(2 + it),
                            qkT[kt_][64:128, jt * 128:(jt + 1) * 128],
                            qkT[qt][64:128, it * 512:(it + 1) * 512],
                            start=True, stop=True, tile_position=(64, 0))
                    et = et_pool.tile([128, 1024], BF16, tag="et", name="et")
                    ets = et_pool.tile([128, 1024], BF16, tag="ets",
                                       name="ets")
                    # exp: DVE does one head via the Schraudolph bit-trick,
                    # ACT the other head; per-chunk ops release each sim
                    # bank as soon as its half is read
                    for it in range(2):
                        nc.vector.tensor_scalar(
                            ets[:, it * 512:(it + 1) * 512].bitcast(I16),
                            sD[it][:], A16, B16, ALU.mult, ALU.add)
                        nc.scalar.activation(et[:, it * 512:(it + 1) * 512],
                                             sA[it][:], AF.Exp)
                    # av/den of jt emit one period late so the in-order PE
                    # queue always has ready work while exp(jt) runs
                    pend.append((lambda j=jt, c=ch, e=et, es=ets:
                                 avden(j, c, e, es)))
                    if len(pend) > 3:
                        pend.pop(0)()
                pend.append(finish)

            # pair-interleaved, but pair0's last group runs two groups
            # before the end so C0 can weave into B's tail
            for pi, hp in [(0, 0), (1, 0), (0, 1), (1, 1), (0, 2), (0, 3),
                           (1, 2), (1, 3)]:
                group(pi, hp)
            while pend:
                pend.pop(0)()

        def emit_C(pi, tags, evict):
            # Two passes: kt0-2 partial chains run as soon as the first
            # three head-pairs' otn exist (weavable into B's tail); only
            # one matmul + one DVE add per token block depends on the
            # last head-pair, shrinking the end-of-kernel serial tail.
            otn = S[pi]['otn']
            part = []
            for tt in range(8):
                tg = tags[tt % len(tags)]
                py = ps.tile([128, 512], F32, tag=tg, name="py")
                if with_bias:
                    nc.tensor.matmul(py[:], ones_b[0:1, :], bout_sb[:],
                                     start=True, stop=False,
                                     tile_position=(0, 0))
                for kt in range(3):
                    nc.tensor.matmul(
                        py[:],
                        otn[kt][:, tt * 128:(tt + 1) * 128],
                        wout_sb[kt][:],
                        start=(not with_bias and kt == 0), stop=(kt == 2))
                yp = y_pool.tile([128, 512], F32, tag="yp", name="yp",
                                 bufs=8)
                # all evictions on DVE: the ACT queue at the tail carries
                # the y DMAs, and an eviction queued behind them stalls
                # every 4th chain's PSUM bank reuse (this variant traces
                # with no PE gap above 1.7us anywhere in the kernel)
                evict(yp[:], py[:])
                part.append(yp)
            for tt in range(8):
                tg = tags[tt % len(tags)]
                py = ps.tile([128, 512], F32, tag=tg, name="py")
                nc.tensor.matmul(
                    py[:],
                    otn[3][:, tt * 128:(tt + 1) * 128],
                    wout_sb[3][:],
                    start=True, stop=True)
                ysb = y_pool.tile([128, 512], F32, tag="ysb", name="ysb",
                                  bufs=8)
                nc.vector.tensor_tensor(ysb[:], part[tt][:], py[:], ALU.add)
                # y rides the ACT HWDGE: the sync queue may still be
                # draining den bounces that wait multi-us on reciprocals
                nc.scalar.dma_start(
                    y.ap()[pi * N + tt * 128:pi * N + (tt + 1) * 128, :],
                    ysb[:])

        def ev_act(dst, src):
            nc.scalar.copy(dst, src)

        def ev_dve(dst, src):
            nc.vector.tensor_copy(dst, src)

        # Both projections run up-front at full PE density (B-phase PE has
        # no slack to absorb them); B is then ACT/PE balanced, with both
        # pairs' head-pair groups interleaved in one phase.
        emit_A(0, ATAGS)
        emit_A(1, ATAGS)
        emit_B()
        emit_C(0, ATAGS, ev_dve)       # starts inside B's tail
        emit_C(1, ATAGS, ev_dve)       # tail: all banks free again

    _orig = nc.to_json_bytes
    nc.to_json_bytes = lambda: _legalize_waits(_orig())
    return nc


_NC_CACHE = []
_last_in_maps = None


def kernel(**inputs) -> np.ndarray:
    import ml_dtypes
    BF = ml_dtypes.bfloat16
    x = np.ascontiguousarray(np.asarray(inputs["x"], dtype=np.float32))
    W_qkv = np.asarray(inputs["W_qkv"], dtype=np.float32)
    W_out = np.ascontiguousarray(np.asarray(inputs["W_out"], dtype=np.float32))
    b_out = np.ascontiguousarray(np.asarray(inputs["b_out"], dtype=np.float32))
    f = int(np.asarray(inputs["f"]))
    assert f == F and x.shape == (B, NTOT, DIM)

    Wqk = np.ascontiguousarray(np.concatenate(
        [W_qkv[:, :512] * SCALE, W_qkv[:, 512:1024]], axis=1).astype(BF))
    Wv = np.ascontiguousarray(W_qkv[:, 1024:1536].astype(BF))
    Wo = np.ascontiguousarray(W_out.astype(BF))
    bo = np.ascontiguousarray(b_out.reshape(1, 512).astype(BF))

    with_bias = bool(np.any(b_out))
    key = with_bias
    if not _NC_CACHE or _NC_CACHE[0][0] != key:
        _NC_CACHE.clear()
        _NC_CACHE.append((key, build(with_bias)))
    nc = _NC_CACHE[0][1]

    in_maps = []
    for core in range(8):
        pairs = (2 * core, 2 * core + 1)
        xT = np.concatenate(
            [x[p // F, (p % F) * N:(p % F + 1) * N, :].T for p in pairs], axis=1)
        in_maps.append({
            "xt": np.ascontiguousarray(xT.astype(BF)),
            "wqk": Wqk, "wv": Wv, "wout": Wo, "bout": bo,
        })

    global _last_in_maps
    _last_in_maps = in_maps
    try:
        res = run_bass_kernel_spmd(nc, in_maps, list(range(8)))
    except Exception:
        # transient NRT_EXEC_UNIT_UNRECOVERABLE occasionally hits the first
        # submission after a fresh compile; one retry has always cleared it
        import time
        time.sleep(10)
        res = run_bass_kernel_spmd(nc, in_maps, list(range(8)))

    out = np.zeros((B, NTOT, DIM), dtype=np.float32)
    for core in range(8):
        yc = res.results[core]["y"]
        for pi, p in enumerate((2 * core, 2 * core + 1)):
            out[p // F, (p % F) * N:(p % F + 1) * N, :] = yc[pi * N:(pi + 1) * N]
    return out

